# revision 1
# baseline (speedup 1.0000x reference)
"""Trainium2 Bass kernel for batched multi-head attention.

Full module:  out = softmax((X_q Wq)(X_k Wk)^T / sqrt(dh) + keymask) (X_v Wv) * qmask
Shapes: B=4, S=2048, D=1024, H=16, dh=64.

Sharding over 8 NeuronCores: core c -> (batch b = c//2, head-group g = c%2).
Each core computes batch b, heads g*8..g*8+8 (Wq/Wk/Wv column-sharded by head).
No collectives; the host scatters inputs and gathers the [2048, 512] output
blocks into the full [4, 2048, 1024] output.

Per-core dataflow (all matmuls in float32r -> full PE rate at N>=256):
  1. PE-transpose X_q/X_k/X_v tiles to X^T (contraction dim on partitions).
  2. Projections: QW^T,KW^T = (W chunks)^T stationary x X^T moving -> [m, s];
     VW = (X^T chunks) stationary x W moving -> [s, m] (natural), stored with
     a ones-column appended per head for free softmax denominators.
  3. Per head h, q-half qh (softmax-pipelined over 16 k-chunks):
       S^T(kc) = KW^T_chunk^T @ QW^T      (K=64 matmul, auto 64x128 array tile)
       P(kc)   = exp(S^T * 0.125 + vbias) (ScalarE, mask+scale fused)
       O^T    += [VW|1]^T @ P(kc)         (K=128, accumulated in PSUM)
     Then PE-transpose the [65, q] O^T block (row 64 = sum of exp), and
     normalize out = O * (qmask/denom) on VectorE.
"""

import os
import sys
import time
import threading

for _p in ("/opt/trn_rl_repo", "/opt/pypackages"):
    if _p not in sys.path and os.path.isdir(_p):
        sys.path.append(_p)

import numpy as np
from contextlib import ExitStack

import concourse.bass as bass
import concourse.tile as tile
from concourse import bacc, mybir
from concourse.bass_utils import run_bass_kernel_spmd
from concourse.masks import make_identity

B, S, D = 4, 2048, 1024
HEADS, DH = 16, 64
NEG_BIG = 1e10
N_CORES = 8
HG = HEADS // 2          # 8 heads per core
MC = HG * DH             # 512 output cols per core
NSC = S // 128           # 16 seq chunks
NDC = D // 128           # 8 contraction chunks
NMC = MC // 128          # 4 head-dim chunks (of this core's 512 cols)
NKC = NSC                # 16 key chunks
NQH = 2                  # q halves
QH = S // NQH            # 1024

F32 = mybir.dt.float32
F32R = mybir.dt.float32r
EXP = mybir.ActivationFunctionType.Exp

# "k128": AV as one K=128 matmul (array mode switches 64<->128 per k-chunk)
# "k64" : AV split into two K=64 matmuls on array tiles (0,0)/(64,0) -> the
#         whole attention loop stays in 64x128 row-tiled mode.
AV_MODE = os.environ.get("AV_MODE", "k64")
N_FILLER = int(os.environ.get("N_FILLER", "0"))


def _r(ap):
    """reinterpret an fp32 AP as float32r for full-rate PE matmul"""
    return ap.bitcast(F32R)


def _emit(tc, t):
    nc = tc.nc
    ctx = ExitStack()

    # ---------------- persistent pools ----------------
    cpool = ctx.enter_context(tc.tile_pool(name="const", bufs=1))
    # prefetch the first X tiles before anything else so the transpose
    # pipeline starts as early as possible
    xq_dram = t["xq"].ap().rearrange("(sc p) d -> sc p d", p=128)
    pre_pool = ctx.enter_context(tc.tile_pool(name="pre", bufs=1))
    pre_x = []
    for i in range(4):
        xpre = pre_pool.tile([128, D], F32R, name=f"xpre{i}", tag=f"xpre{i}")
        nc.sync.dma_start(xpre[:], xq_dram[i])
        pre_x.append(xpre)

    ident = cpool.tile([128, 128], F32)
    make_identity(nc, ident[:])
    ident_r = cpool.tile([128, 128], F32R)
    nc.vector.tensor_copy(ident_r[:], ident[:])
    vbias = cpool.tile([128, NKC], F32)
    nc.sync.dma_start(vbias[:], t["vbias"].ap())
    qmaskT = cpool.tile([128, NSC], F32)
    nc.sync.dma_start(qmaskT[:], t["qmaskT"].ap())

    scratch_bf = cpool.tile([128, 128], mybir.dt.bfloat16)
    nc.vector.memset(scratch_bf[:], 0.0)

    qk_pool = ctx.enter_context(tc.tile_pool(name="qk", bufs=1))
    qwT = qk_pool.tile([128, NMC, S], F32R)        # [m%128, mc, s] 32KB/part
    kwT = qk_pool.tile([128, NMC, S], F32R)
    vw = qk_pool.tile([128, NKC, HG, DH + 1], F32R)  # [k%128, kc, h, dh|1]
    ones = cpool.tile([128, 1], F32)
    nc.vector.memset(ones[:], 1.0)
    nc.vector.tensor_copy(                           # denominator ones column
        vw[:, :, :, DH:DH + 1], ones[:].broadcast_to([128, NKC, HG, 1])
    )

    # ---------------- projection phase ----------------
    pctx = ExitStack()
    xt_pool = pctx.enter_context(tc.tile_pool(name="xt", bufs=1))
    x_pool = pctx.enter_context(tc.tile_pool(name="x", bufs=6))
    w_pool = pctx.enter_context(tc.tile_pool(name="w", bufs=2))
    psum_t = pctx.enter_context(tc.tile_pool(name="ps_t", bufs=2, space="PSUM"))
    psum_p = pctx.enter_context(tc.tile_pool(name="ps_p", bufs=2, space="PSUM"))

    HSC = NSC // 2  # s-chunks per half

    for xi, (xname, kind) in enumerate((("xq", "q"), ("xk", "k"), ("xv", "v"))):
        x_dram = t[xname].ap().rearrange("(sc p) d -> sc p d", p=128)
        w_dram = t["w" + kind].ap().rearrange("(dc p) m -> p dc m", p=128)
        w_sb = w_pool.tile([128, NDC, MC], F32R, tag="w")
        nc.sync.dma_start(w_sb[:], w_dram)

        for sh in range(2):  # s-halves
            # transpose this half of X into xt [d%128, dc, s_local]
            xt = xt_pool.tile([128, NDC, QH], F32R, tag="xt")
            for scl in range(HSC):
                sc = sh * HSC + scl
                if xname == "xq" and sh == 0 and scl < len(pre_x):
                    xt_in = pre_x[scl]
                else:
                    xt_in = x_pool.tile([128, D], F32R, tag="x")
                    nc.sync.dma_start(xt_in[:], x_dram[sc])
                pt = psum_t.tile([128, NDC, 128], F32R, tag="pt")
                for dc in range(NDC):
                    nc.tensor.transpose(
                        pt[:, dc, :], xt_in[:, dc * 128:(dc + 1) * 128], ident_r[:]
                    )
                if scl % 2 == 0:
                    nc.vector.tensor_copy(xt[:, :, scl * 128:(scl + 1) * 128], pt[:])
                else:
                    nc.scalar.copy(xt[:, :, scl * 128:(scl + 1) * 128], pt[:])

            if kind in ("q", "k"):
                dst = qwT if kind == "q" else kwT
                for mc in range(NMC):
                    pp = psum_p.tile([128, QH], F32, tag="pp")
                    for dc in range(NDC):
                        for nh in range(QH // 512):
                            nc.tensor.matmul(
                                pp[:, nh * 512:(nh + 1) * 512],
                                w_sb[:, dc, mc * 128:(mc + 1) * 128],
                                xt[:, dc, nh * 512:(nh + 1) * 512],
                                start=(dc == 0),
                                stop=(dc == NDC - 1),
                            )
                    nc.vector.tensor_copy(
                        dst[:, mc, sh * QH:(sh + 1) * QH], pp[:]
                    )
            else:
                for scl in range(HSC):
                    sc = sh * HSC + scl
                    pv = psum_p.tile([128, MC], F32, tag="pp")
                    for dc in range(NDC):
                        nc.tensor.matmul(
                            pv[:],
                            xt[:, dc, scl * 128:(scl + 1) * 128],
                            w_sb[:, dc, :],
                            start=(dc == 0),
                            stop=(dc == NDC - 1),
                        )
                    nc.vector.tensor_copy(
                        vw[:, sc, :, 0:DH],
                        pv[:].rearrange("p (h d) -> p h d", h=HG),
                    )

    pctx.close()

    # ---------------- attention phase ----------------
    actx = ExitStack()
    p_pool = actx.enter_context(tc.tile_pool(name="p", bufs=3))
    ot_pool = actx.enter_context(tc.tile_pool(name="ot", bufs=2))
    rq_pool = actx.enter_context(tc.tile_pool(name="rq", bufs=2))
    out_pool = actx.enter_context(tc.tile_pool(name="out", bufs=3))
    psum_s = actx.enter_context(tc.tile_pool(name="ps_s", bufs=2, space="PSUM"))
    psum_o = actx.enter_context(tc.tile_pool(name="ps_o", bufs=4, space="PSUM"))

    # DRAM view: [qh, p, qb, h, d] for per-(head, q-half) strip stores
    out_v = t["out"].ap().rearrange(
        "(a qb p) (hh d) -> a p qb hh d", a=NQH, p=128, hh=HG
    )

    def filler(n):
        for _ in range(n):
            nc.tensor.ldweights(scratch_bf[:])

    pending_tail = [None]

    for h in range(HG):
        mc_h = h // 2
        p0 = (h % 2) * 64
        kw_h = kwT[p0:p0 + 64, mc_h, :]
        qw_h = qwT[p0:p0 + 64, mc_h, :]
        for qh in range(NQH):
            q0 = qh * QH
            o_lo = []
            o_hi = []
            for nh in range(QH // 512):
                ol = psum_o.tile([DH + 1, 512], F32, tag="o", name=f"olo{h}_{qh}_{nh}")
                o_lo.append(ol)
            for nh in range(QH // 512):
                oh = psum_o.tile([DH + 1, 512], F32, tag="o", name=f"ohi{h}_{qh}_{nh}")
                o_hi.append(oh)

            def emit_s(kc):
                s_ps = psum_s.tile([128, QH], F32, tag="s")
                for nh in range(QH // 512):
                    nc.tensor.matmul(
                        s_ps[:, nh * 512:(nh + 1) * 512],
                        kw_h[:, kc * 128:(kc + 1) * 128],
                        qw_h[:, q0 + nh * 512:q0 + (nh + 1) * 512],
                        start=True,
                        stop=True,
                    )
                return s_ps

            def emit_exp(kc, s_ps):
                p_t = p_pool.tile([128, QH], F32R, tag="p")
                nc.scalar.activation(
                    p_t[:], s_ps[:], EXP,
                    bias=vbias[:, kc:kc + 1], scale=0.125,
                )
                return p_t

            def emit_av(kc, p_t):
                first, last = kc == 0, kc == NKC - 1
                for nh in range(QH // 512):
                    psl = p_t[:, nh * 512:(nh + 1) * 512]
                    # two K=64 halves on array tiles (0,0)/(64,0); they can
                    # run concurrently, so they need separate PSUM regions
                    nc.tensor.matmul(
                        o_lo[nh][:], vw[0:64, kc, h, :], psl[0:64, :],
                        start=first, stop=last,
                    )
                    nc.tensor.matmul(
                        o_hi[nh][:], vw[64:128, kc, h, :], psl[64:128, :],
                        start=first, stop=last,
                    )

            # software pipeline: keep PE one S-matmul ahead of ACT's exp.
            # The previous iteration's evacuate/transpose/normalize tail is
            # emitted after this iteration's first two S matmuls so it
            # overlaps the new exp stream instead of stalling it.
            s_prev = emit_s(0)
            s_cur = emit_s(1)
            for kc in range(1, NKC):
                p_t = emit_exp(kc - 1, s_prev)
                if kc == 6 and pending_tail[0] is not None:
                    pending_tail[0]()
                filler(N_FILLER)
                emit_av(kc - 1, p_t)
                s_prev = s_cur
                s_cur = emit_s(kc + 1) if kc + 1 < NKC else None
            p_t = emit_exp(NKC - 1, s_prev)
            filler(N_FILLER)
            emit_av(NKC - 1, p_t)

            def make_tail(h=h, qh=qh, o_lo=o_lo, o_hi=o_hi):
                def tail():
                    # evacuate O^T halves, transpose [65,128] blocks -> [128, 65]
                    ot = ot_pool.tile([DH + 1, QH], F32, tag="ot", name=f"ot_{h}_{qh}")
                    for nh in range(QH // 512):
                        osl = ot[:, nh * 512:(nh + 1) * 512]
                        nc.vector.tensor_copy(osl, o_lo[nh][:])
                        nc.vector.tensor_add(osl, osl, o_hi[nh][:])
                    tr = psum_s.tile([128, 8, DH + 1], F32, tag="s", name=f"tr_{h}_{qh}")
                    for qb in range(8):
                        nc.tensor.transpose(
                            tr[:, qb, :],
                            ot[:, qb * 128:(qb + 1) * 128],
                            ident[0:DH + 1, 0:DH + 1],
                        )
                    # normalize: out = O * qmask/denom (denom = col 64)
                    rq = rq_pool.tile([128, 8], F32, tag="rq", name=f"rq_{h}_{qh}")
                    nc.vector.reciprocal(rq[:], tr[:, :, DH])
                    nc.vector.tensor_mul(
                        rq[:], rq[:], qmaskT[:, qh * 8:(qh + 1) * 8]
                    )
                    ob = out_pool.tile([128, 8, DH], F32, tag="ob", name=f"ob_{h}_{qh}")
                    nc.vector.tensor_mul(
                        ob[:], tr[:, :, 0:DH], rq[:].broadcast_to([128, 8, DH])
                    )
                    nc.sync.dma_start(out_v[qh][:, :, h, :], ob[:])
                return tail

            pending_tail[0] = make_tail()

    pending_tail[0]()
    actx.close()
    ctx.close()


_BUILD_LOCK = threading.Lock()
_CACHE = {}


def _build():
    with _BUILD_LOCK:
        if "nc" in _CACHE:
            return _CACHE["nc"]
        nc = bacc.Bacc(
            "TRN2", target_bir_lowering=False, debug=False, num_devices=N_CORES
        )
        t = {
            "xq": nc.dram_tensor("xq", [S, D], F32R, kind="ExternalInput"),
            "xk": nc.dram_tensor("xk", [S, D], F32R, kind="ExternalInput"),
            "xv": nc.dram_tensor("xv", [S, D], F32R, kind="ExternalInput"),
            "wq": nc.dram_tensor("wq", [D, MC], F32R, kind="ExternalInput"),
            "wk": nc.dram_tensor("wk", [D, MC], F32R, kind="ExternalInput"),
            "wv": nc.dram_tensor("wv", [D, MC], F32R, kind="ExternalInput"),
            "vbias": nc.dram_tensor("vbias", [128, NKC], F32, kind="ExternalInput"),
            "qmaskT": nc.dram_tensor("qmaskT", [128, NSC], F32, kind="ExternalInput"),
            "out": nc.dram_tensor("out", [S, MC], F32, kind="ExternalOutput"),
        }
        with tile.TileContext(nc) as tc:
            _emit(tc, t)
        nc.compile()
        _CACHE["nc"] = nc
        return nc


def _in_maps(q_value, k_value, v_value, v_mask, q_mask, Wq, Wk, Wv):
    maps = []
    for c in range(N_CORES):
        b, g = c // 2, c % 2
        m0 = g * MC
        vb = ((v_mask[b, :, 0].reshape(NKC, 128).T) - 1.0) * NEG_BIG
        qm = q_mask[b, :, 0].reshape(NSC, 128).T
        maps.append({
            "xq": np.ascontiguousarray(q_value[b]),
            "xk": np.ascontiguousarray(k_value[b]),
            "xv": np.ascontiguousarray(v_value[b]),
            "wq": np.ascontiguousarray(Wq[:, m0:m0 + MC]),
            "wk": np.ascontiguousarray(Wk[:, m0:m0 + MC]),
            "wv": np.ascontiguousarray(Wv[:, m0:m0 + MC]),
            "vbias": np.ascontiguousarray(vb).astype(np.float32),
            "qmaskT": np.ascontiguousarray(qm).astype(np.float32),
        })
    return maps


def _assemble(results):
    out = np.empty((B, S, HEADS * DH), dtype=np.float32)
    for c in range(N_CORES):
        b, g = c // 2, c % 2
        out[b, :, g * MC:(g + 1) * MC] = results[c]["out"]
    return out


def kernel(q_value, k_value, v_value, v_mask, q_mask, Wq, Wk, Wv,
           profile=False, trace_cores=None):
    nc = _build()
    maps = _in_maps(np.asarray(q_value, dtype=np.float32),
                    np.asarray(k_value, dtype=np.float32),
                    np.asarray(v_value, dtype=np.float32),
                    np.asarray(v_mask, dtype=np.float32),
                    np.asarray(q_mask, dtype=np.float32),
                    np.asarray(Wq, dtype=np.float32),
                    np.asarray(Wk, dtype=np.float32),
                    np.asarray(Wv, dtype=np.float32))
    if profile:
        _install_profile_hook()
    res = run_bass_kernel_spmd(
        nc, maps, list(range(N_CORES)),
        trace=profile, trace_cores=trace_cores,
    )
    out = _assemble(res.results)
    if profile:
        return out, res
    return out


def _install_profile_hook():
    """Wire up the NTFF profile hook that this container image lacks."""
    import types
    if "antenv.axon_hooks" in sys.modules:
        return
    try:
        from trn_agent_boot.trn_boot import _ntff_profile_via_ctypes
        hook = _ntff_profile_via_ctypes("/opt/axon/libaxon_pjrt.so")
    except Exception:
        hook = None
    mod = types.ModuleType("antenv.axon_hooks")
    mod.get_axon_ntff_profile_hook = lambda: hook
    sys.modules["antenv.axon_hooks"] = mod


if __name__ == "__main__":
    t0 = time.time()
    _build()
    print(f"build+compile: {time.time() - t0:.1f}s")



# revision 8
# speedup vs baseline: 1.4556x; 1.4556x over previous
"""Trainium2 Bass kernel for batched multi-head attention (v2, all-bf16).

Full module:  out = softmax((X_q Wq)(X_k Wk)^T / sqrt(dh) + keymask) (X_v Wv) * qmask
Shapes: B=4, S=2048, D=1024, H=16, dh=64.

Sharding over 8 NeuronCores: core c -> (batch b = c//2, head-group g = c%2).
Each core computes batch b, heads g*8..g*8+8 (Wq/Wk/Wv column-sharded by head).
No collectives; the host scatters inputs and gathers the [2048, 512] output
blocks into the full [4, 2048, 1024] output.

Host-side marshaling: X tensors are transposed (X^T, contraction dim on
partitions) and cast to bf16; W column blocks cast to bf16; v_mask is folded
into X_v rows (numerator) and shipped as vmaskT (denominator column). This
removes all on-chip PE transposes of X and their PSUM evacuations.

Per-core schedule (all matmuls bf16, moving N=512):
  Phase 0: V projection + Q/K projections for head pair 0 (mc=0).
  Attention, one head PAIR at a time (heads 2i/2i+1 live on partition halves
  0:64 / 64:128 of the mc=i chunk of QW^T/KW^T):
    per kc: S^T for both heads -> one [128, 2, 512] PSUM tile via two
    CONCURRENT K=64 matmuls on PE array row-tiles (0,0)/(64,0);
    one ScalarE exp (N=1024, bf16 out) covers both heads;
    two K=128 AV matmuls accumulate O^T[65, 512] per head (row 64 = sum of
    exp * v_mask = softmax denominator).
  The exp stream is the bottleneck (~1.1us per kc); leftover PE time inside
  the loop is filled with the NEXT head pair's Q/K projection matmuls
  (pulled from a generator), so projections cost almost no wall time.
  Tails (PE-transpose O^T, normalize by qmask/denom, DMA out) are deferred
  into the next iteration's stream.
"""

import os
import sys
import time
import threading

for _p in ("/opt/trn_rl_repo", "/opt/pypackages"):
    if _p not in sys.path and os.path.isdir(_p):
        sys.path.append(_p)

import numpy as np
import ml_dtypes
from contextlib import ExitStack

import concourse.bass as bass
import concourse.tile as tile
from concourse import bacc, mybir
from concourse.bass_utils import run_bass_kernel_spmd
from concourse.masks import make_identity

B, S, D = 4, 2048, 1024
HEADS, DH = 16, 64
N_CORES = 8
HG = HEADS // 2          # 8 heads per core
MC = HG * DH             # 512 output cols per core
NSC = S // 128           # 16 seq chunks
NDC = D // 128           # 8 contraction chunks
NMC = MC // 128          # 4 head-dim chunks (= head pairs)
NKC = NSC                # 16 key chunks

F32 = mybir.dt.float32
BF16 = mybir.dt.bfloat16
EXP = mybir.ActivationFunctionType.Exp

QH = 512                 # q-half size
NQH = S // QH
QB = QH // 128
N_FILL = int(os.environ.get("N_FILL", "2"))   # filler units pulled per kc


def _emit(tc, t):
    nc = tc.nc
    ctx = ExitStack()

    # ---------------- persistent pools / DMAs ----------------
    cpool = ctx.enter_context(tc.tile_pool(name="const", bufs=1))
    x_pool = ctx.enter_context(tc.tile_pool(name="x", bufs=1))
    w_pool = ctx.enter_context(tc.tile_pool(name="w", bufs=1))

    # X^T inputs [d%128, dc, s] bf16 (xv first: V projection runs first)
    xts = {}
    for name in ("xv", "xq", "xk"):
        xt = x_pool.tile([128, NDC, S], BF16, name=name + "t", tag=name + "t")
        nc.sync.dma_start(xt[:], t[name].ap().rearrange("(dc p) s -> p dc s", p=128))
        xts[name] = xt
    w_sbs = {}
    for kind in ("v", "q", "k"):
        w_sb = w_pool.tile([128, NDC, MC], BF16, name="w" + kind, tag="w" + kind)
        nc.sync.dma_start(w_sb[:], t["w" + kind].ap().rearrange("(dc p) m -> p dc m", p=128))
        w_sbs[kind] = w_sb

    ident = cpool.tile([128, 128], F32)
    make_identity(nc, ident[:])
    zbias = cpool.tile([128, 1], F32)
    nc.vector.memset(zbias[:], 0.0)
    qmaskT = cpool.tile([128, NSC], F32)
    nc.sync.dma_start(qmaskT[:], t["qmaskT"].ap())
    vmaskT = cpool.tile([128, NKC], BF16)
    nc.sync.dma_start(vmaskT[:], t["vmaskT"].ap())

    # exp table warmup while DMAs stream
    warm = cpool.tile([128, 1], BF16)
    nc.scalar.activation(warm[:], zbias[:], EXP, bias=zbias[:], scale=1.0)

    qk_pool = ctx.enter_context(tc.tile_pool(name="qk", bufs=1))
    qwT = qk_pool.tile([128, NMC, S], BF16)      # [m%128, mc, s]
    kwT = qk_pool.tile([128, NMC, S], BF16)
    vw = qk_pool.tile([128, NKC, HG, DH + 1], BF16)  # [k%128, kc, h, dh|m]
    nc.vector.tensor_copy(
        vw[:, :, :, DH:DH + 1],
        vmaskT[:].rearrange("p (k a b) -> p k a b", a=1, b=1)
        .broadcast_to([128, NKC, HG, 1]),
    )

    # ---------------- phase 0: V proj + Q/K mc=0 ----------------
    pctx = ExitStack()
    psum_p = pctx.enter_context(tc.tile_pool(name="ps_p", bufs=2, space="PSUM"))

    for sc in range(NSC):
        pv = psum_p.tile([128, MC], F32, tag="pp")
        for dc in range(NDC):
            nc.tensor.matmul(
                pv[:], xts["xv"][:, dc, sc * 128:(sc + 1) * 128],
                w_sbs["v"][:, dc, :],
                start=(dc == 0), stop=(dc == NDC - 1),
            )
        view = pv[:].rearrange("p (h d) -> p h d", h=HG)
        if sc % 2 == 0:
            nc.vector.tensor_copy(vw[:, sc, :, 0:DH], view)
        else:
            nc.scalar.copy(vw[:, sc, :, 0:DH], view)

    def emit_qk_group(kind, mc, sq, pool, n=None):
        """one [128,512] psum group of the Q/K projection; generator-style"""
        dst = qwT if kind == "q" else kwT
        pp = pool.tile([128, 512], F32, tag="f")
        for dc in range(NDC):
            nc.tensor.matmul(
                pp[:], w_sbs[kind][:, dc, mc * 128:(mc + 1) * 128],
                xts["x" + kind][:, dc, sq * 512:(sq + 1) * 512],
                start=(dc == 0), stop=(dc == NDC - 1),
            )
            if n is not None:
                yield
        if kind == "q" and sq % 2 == 0:
            nc.scalar.copy(dst[:, mc, sq * 512:(sq + 1) * 512], pp[:])
        else:
            nc.vector.tensor_copy(dst[:, mc, sq * 512:(sq + 1) * 512], pp[:])
        if n is not None:
            yield

    for kind in ("q", "k"):
        for sq in range(4):
            for _ in emit_qk_group(kind, 0, sq, psum_p, n=None) or ():
                pass

    pctx.close()

    # ---------------- attention (+ projection fillers) ----------------
    actx = ExitStack()
    p_pool = actx.enter_context(tc.tile_pool(name="p", bufs=3))
    ot_pool = actx.enter_context(tc.tile_pool(name="ot", bufs=4))
    rq_pool = actx.enter_context(tc.tile_pool(name="rq", bufs=2))
    out_pool = actx.enter_context(tc.tile_pool(name="out", bufs=4))
    psum_s = actx.enter_context(tc.tile_pool(name="ps_s", bufs=2, space="PSUM"))
    psum_o = actx.enter_context(tc.tile_pool(name="ps_o", bufs=2, space="PSUM"))
    psum_f = actx.enter_context(tc.tile_pool(name="ps_f", bufs=2, space="PSUM"))

    def filler_gen():
        for mc in (1, 2, 3):
            for kind in ("q", "k"):
                for sq in range(4):
                    yield from emit_qk_group(kind, mc, sq, psum_f, n=1)

    fill = filler_gen()
    pulled = [0]
    UNITS_PER_MC = 2 * 4 * (NDC + 1)   # q+k, 4 sq-groups, 8 matmuls + evac

    def pull(n):
        for _ in range(n):
            if next(fill, "done") == "done":
                break
            pulled[0] += 1

    out_v = t["out"].ap().rearrange(
        "(a qb p) (hh d) -> a p qb hh d", a=NQH, p=128, hh=HG)

    pending_tail = [None]

    for hp in range(NMC):
        # the qwT/kwT chunk this head pair reads must be fully emitted first
        pull(max(0, UNITS_PER_MC * hp - pulled[0]))
        kw_lo = kwT[0:64, hp, :]
        kw_hi = kwT[64:128, hp, :]
        qw_lo = qwT[0:64, hp, :]
        qw_hi = qwT[64:128, hp, :]
        for qh in range(NQH):
            q0 = qh * QH
            o_lo = psum_o.tile([DH + 1, QH], F32, tag="o", name=f"olo{hp}_{qh}")
            o_hi = psum_o.tile([DH + 1, QH], F32, tag="o", name=f"ohi{hp}_{qh}")

            def emit_s(kc):
                s_ps = psum_s.tile([128, 2, QH], F32, tag="s")
                nc.tensor.matmul(
                    s_ps[:, 0, :], kw_lo[:, kc * 128:(kc + 1) * 128],
                    qw_lo[:, q0:q0 + QH], start=True, stop=True)
                nc.tensor.matmul(
                    s_ps[:, 1, :], kw_hi[:, kc * 128:(kc + 1) * 128],
                    qw_hi[:, q0:q0 + QH], start=True, stop=True)
                return s_ps

            def emit_exp(s_ps):
                p_t = p_pool.tile([128, 2, QH], BF16, tag="p")
                nc.scalar.activation(
                    p_t[:], s_ps[:], EXP, bias=zbias[:], scale=0.125)
                return p_t

            def emit_av(kc, p_t):
                first, last = kc == 0, kc == NKC - 1
                nc.tensor.matmul(o_lo[:], vw[:, kc, 2 * hp, :], p_t[:, 0, :],
                                 start=first, stop=last)
                nc.tensor.matmul(o_hi[:], vw[:, kc, 2 * hp + 1, :], p_t[:, 1, :],
                                 start=first, stop=last)

            s_prev = emit_s(0)
            s_cur = emit_s(1)
            for kc in range(NKC):
                p_t = emit_exp(s_prev)
                if kc == 3 and pending_tail[0] is not None:
                    pending_tail[0]()
                    pending_tail[0] = None
                pull(N_FILL)
                emit_av(kc, p_t)
                s_prev = s_cur
                s_cur = emit_s(kc + 2) if kc + 2 < NKC else None
            if pending_tail[0] is not None:
                pending_tail[0]()

            def make_tail(hp=hp, qh=qh, o_lo=o_lo, o_hi=o_hi):
                def one(h, o_ps):
                    ot = ot_pool.tile([DH + 1, QH], F32, tag="ot",
                                      name=f"ot_{h}_{qh}")
                    nc.vector.tensor_copy(ot[:], o_ps[:])
                    tr = psum_f.tile([128, QB, DH + 1], F32, tag="f",
                                     name=f"tr_{h}_{qh}")
                    for qb in range(QB):
                        nc.tensor.transpose(
                            tr[:, qb, :], ot[:, qb * 128:(qb + 1) * 128],
                            ident[0:DH + 1, 0:DH + 1])
                    rq = rq_pool.tile([128, QB], F32, tag="rq",
                                      name=f"rq_{h}_{qh}")
                    nc.vector.reciprocal(rq[:], tr[:, :, DH])
                    nc.vector.tensor_mul(
                        rq[:], rq[:], qmaskT[:, qh * QB:(qh + 1) * QB])
                    ob = out_pool.tile([128, QB, DH], F32, tag="ob",
                                       name=f"ob_{h}_{qh}")
                    nc.vector.tensor_mul(
                        ob[:], tr[:, :, 0:DH], rq[:].broadcast_to([128, QB, DH]))
                    nc.sync.dma_start(out_v[qh][:, :, h, :], ob[:])

                def tail():
                    one(2 * hp, o_lo)
                    one(2 * hp + 1, o_hi)
                return tail

            pending_tail[0] = make_tail()

    pending_tail[0]()
    pull(10 ** 9)
    actx.close()
    ctx.close()


_BUILD_LOCK = threading.Lock()
_CACHE = {}


def _build():
    with _BUILD_LOCK:
        if "nc" in _CACHE:
            return _CACHE["nc"]
        nc = bacc.Bacc(
            "TRN2", target_bir_lowering=False, debug=False, num_devices=N_CORES
        )
        t = {
            "xq": nc.dram_tensor("xq", [D, S], BF16, kind="ExternalInput"),
            "xk": nc.dram_tensor("xk", [D, S], BF16, kind="ExternalInput"),
            "xv": nc.dram_tensor("xv", [D, S], BF16, kind="ExternalInput"),
            "wq": nc.dram_tensor("wq", [D, MC], BF16, kind="ExternalInput"),
            "wk": nc.dram_tensor("wk", [D, MC], BF16, kind="ExternalInput"),
            "wv": nc.dram_tensor("wv", [D, MC], BF16, kind="ExternalInput"),
            "vmaskT": nc.dram_tensor("vmaskT", [128, NKC], BF16,
                                     kind="ExternalInput"),
            "qmaskT": nc.dram_tensor("qmaskT", [128, NSC], F32,
                                     kind="ExternalInput"),
            "out": nc.dram_tensor("out", [S, MC], F32, kind="ExternalOutput"),
        }
        with tile.TileContext(nc) as tc:
            _emit(tc, t)
        nc.compile()
        _CACHE["nc"] = nc
        return nc


def _in_maps(q_value, k_value, v_value, v_mask, q_mask, Wq, Wk, Wv):
    bf = ml_dtypes.bfloat16
    xqt, xkt, xvt = {}, {}, {}
    for b in range(B):
        xqt[b] = np.ascontiguousarray(q_value[b].T.astype(bf))
        xkt[b] = np.ascontiguousarray(k_value[b].T.astype(bf))
        # fold key mask into the V rows (numerator side)
        xvt[b] = np.ascontiguousarray((v_value[b] * v_mask[b]).T.astype(bf))
    w8 = {}
    for g in range(2):
        m0 = g * MC
        w8[g] = (np.ascontiguousarray(Wq[:, m0:m0 + MC].astype(bf)),
                 np.ascontiguousarray(Wk[:, m0:m0 + MC].astype(bf)),
                 np.ascontiguousarray(Wv[:, m0:m0 + MC].astype(bf)))
    maps = []
    for c in range(N_CORES):
        b, g = c // 2, c % 2
        vm = v_mask[b, :, 0].reshape(NKC, 128).T
        qm = q_mask[b, :, 0].reshape(NSC, 128).T
        maps.append({
            "xq": xqt[b], "xk": xkt[b], "xv": xvt[b],
            "wq": w8[g][0], "wk": w8[g][1], "wv": w8[g][2],
            "vmaskT": np.ascontiguousarray(vm.astype(bf)),
            "qmaskT": np.ascontiguousarray(qm).astype(np.float32),
        })
    return maps


def _assemble(results):
    out = np.empty((B, S, HEADS * DH), dtype=np.float32)
    for c in range(N_CORES):
        b, g = c // 2, c % 2
        out[b, :, g * MC:(g + 1) * MC] = results[c]["out"]
    return out


def kernel(q_value, k_value, v_value, v_mask, q_mask, Wq, Wk, Wv,
           profile=False, trace_cores=None):
    nc = _build()
    maps = _in_maps(np.asarray(q_value, dtype=np.float32),
                    np.asarray(k_value, dtype=np.float32),
                    np.asarray(v_value, dtype=np.float32),
                    np.asarray(v_mask, dtype=np.float32),
                    np.asarray(q_mask, dtype=np.float32),
                    np.asarray(Wq, dtype=np.float32),
                    np.asarray(Wk, dtype=np.float32),
                    np.asarray(Wv, dtype=np.float32))
    if profile:
        _install_profile_hook()
    res = run_bass_kernel_spmd(
        nc, maps, list(range(N_CORES)),
        trace=profile, trace_cores=trace_cores,
    )
    out = _assemble(res.results)
    if profile:
        return out, res
    return out


def _install_profile_hook():
    """Wire up the NTFF profile hook that this container image lacks."""
    import types
    if "antenv.axon_hooks" in sys.modules:
        return
    try:
        from trn_agent_boot.trn_boot import _ntff_profile_via_ctypes
        hook = _ntff_profile_via_ctypes("/opt/axon/libaxon_pjrt.so")
    except Exception:
        hook = None
    mod = types.ModuleType("antenv.axon_hooks")
    mod.get_axon_ntff_profile_hook = lambda: hook
    sys.modules["antenv.axon_hooks"] = mod


if __name__ == "__main__":
    t0 = time.time()
    _build()
    print(f"build+compile: {time.time() - t0:.1f}s")


# revision 14
# speedup vs baseline: 1.5388x; 1.0572x over previous
"""Trainium2 Bass kernel for batched multi-head attention (v2, all-bf16).

Full module:  out = softmax((X_q Wq)(X_k Wk)^T / sqrt(dh) + keymask) (X_v Wv) * qmask
Shapes: B=4, S=2048, D=1024, H=16, dh=64.

Sharding over 8 NeuronCores: core c -> (batch b = c//2, head-group g = c%2).
Each core computes batch b, heads g*8..g*8+8 (Wq/Wk/Wv column-sharded by head).
No collectives; the host scatters inputs and gathers the [2048, 512] output
blocks into the full [4, 2048, 1024] output.

Host-side marshaling: X tensors are transposed (X^T, contraction dim on
partitions) and cast to bf16; W column blocks cast to bf16; v_mask is folded
into X_v rows (numerator) and shipped as vmaskT (denominator column). This
removes all on-chip PE transposes of X and their PSUM evacuations.

Per-core schedule (all matmuls bf16, moving N=512):
  Phase 0: V projection + Q/K projections for head pair 0 (mc=0).
  Attention, one head PAIR at a time (heads 2i/2i+1 live on partition halves
  0:64 / 64:128 of the mc=i chunk of QW^T/KW^T):
    per kc: S^T for both heads -> one [128, 2, 512] PSUM tile via two
    CONCURRENT K=64 matmuls on PE array row-tiles (0,0)/(64,0);
    one ScalarE exp (N=1024, bf16 out) covers both heads;
    two K=128 AV matmuls accumulate O^T[65, 512] per head (row 64 = sum of
    exp * v_mask = softmax denominator).
  The exp stream is the bottleneck (~1.1us per kc); leftover PE time inside
  the loop is filled with the NEXT head pair's Q/K projection matmuls
  (pulled from a generator), so projections cost almost no wall time.
  Tails (PE-transpose O^T, normalize by qmask/denom, DMA out) are deferred
  into the next iteration's stream.
"""

import os
import sys
import time
import threading

for _p in ("/opt/trn_rl_repo", "/opt/pypackages"):
    if _p not in sys.path and os.path.isdir(_p):
        sys.path.append(_p)

import numpy as np
import ml_dtypes
from contextlib import ExitStack

import concourse.bass as bass
import concourse.tile as tile
from concourse import bacc, mybir
from concourse.bass_utils import run_bass_kernel_spmd
from concourse.masks import make_identity

B, S, D = 4, 2048, 1024
HEADS, DH = 16, 64
N_CORES = 8
HG = HEADS // 2          # 8 heads per core
MC = HG * DH             # 512 output cols per core
NSC = S // 128           # 16 seq chunks
NDC = D // 128           # 8 contraction chunks
NMC = MC // 128          # 4 head-dim chunks (= head pairs)
NKC = NSC                # 16 key chunks

F32 = mybir.dt.float32
BF16 = mybir.dt.bfloat16
EXP = mybir.ActivationFunctionType.Exp

QH = 512                 # q-half size
NQH = S // QH
QB = QH // 128
N_FILL = int(os.environ.get("N_FILL", "2"))   # filler units pulled per kc


def _emit(tc, t):
    nc = tc.nc
    ctx = ExitStack()

    # ---------------- persistent pools / DMAs ----------------
    cpool = ctx.enter_context(tc.tile_pool(name="const", bufs=1))
    x_pool = ctx.enter_context(tc.tile_pool(name="x", bufs=1))
    w_pool = ctx.enter_context(tc.tile_pool(name="w", bufs=1))

    # W first (small, needed by the first projections), then X^T quarters in
    # consumption order: xq/xk (phase-0 Q/K mc0) before xv (filler V proj).
    w_sbs = {}
    for kind in ("q", "k", "v"):
        w_sb = w_pool.tile([128, NDC, MC], BF16, name="w" + kind, tag="w" + kind)
        nc.sync.dma_start(w_sb[:], t["w" + kind].ap().rearrange("(dc p) m -> p dc m", p=128))
        w_sbs[kind] = w_sb
    xts = {}
    for name in ("xq", "xk", "xv"):
        xt = x_pool.tile([128, NDC, S], BF16, name=name + "t", tag=name + "t")
        xts[name] = xt
    x_views = {name: t[name].ap().rearrange("(dc p) s -> p dc s", p=128)
               for name in ("xq", "xk", "xv")}
    for name in ("xq", "xk", "xv"):
        for sq in range(4):
            nc.sync.dma_start(xts[name][:, :, sq * 512:(sq + 1) * 512],
                              x_views[name][:, :, sq * 512:(sq + 1) * 512])

    ident = cpool.tile([128, 128], F32)
    make_identity(nc, ident[:])
    zbias = cpool.tile([128, 1], F32)
    nc.vector.memset(zbias[:], 0.0)
    qmaskT = cpool.tile([128, NSC], F32)
    nc.sync.dma_start(qmaskT[:], t["qmaskT"].ap())
    vmaskT = cpool.tile([128, NKC], BF16)
    nc.sync.dma_start(vmaskT[:], t["vmaskT"].ap())

    # exp table warmup while DMAs stream
    warm = cpool.tile([128, 1], BF16)
    nc.scalar.activation(warm[:], zbias[:], EXP, bias=zbias[:], scale=1.0)

    qk_pool = ctx.enter_context(tc.tile_pool(name="qk", bufs=1))
    qwT = qk_pool.tile([128, NMC, S], BF16)      # [m%128, mc, s]
    kwT = qk_pool.tile([128, NMC, S], BF16)
    vw = qk_pool.tile([128, NKC, HG, DH + 1], BF16)  # [k%128, kc, h, dh|m]
    nc.vector.tensor_copy(
        vw[:, :, :, DH:DH + 1],
        vmaskT[:].rearrange("p (k a b) -> p k a b", a=1, b=1)
        .broadcast_to([128, NKC, HG, 1]),
    )

    # ---------------- phase 0: Q/K mc=0 only ----------------
    pctx = ExitStack()
    psum_p = pctx.enter_context(tc.tile_pool(name="ps_p", bufs=2, space="PSUM"))

    def emit_v_group(sc, pool):
        """one [128,512] psum group of the V projection; generator-style"""
        pv = pool.tile([128, MC], F32, tag="f")
        for dc in range(NDC):
            nc.tensor.matmul(
                pv[:], xts["xv"][:, dc, sc * 128:(sc + 1) * 128],
                w_sbs["v"][:, dc, :],
                start=(dc == 0), stop=(dc == NDC - 1),
            )
            yield
        nc.vector.tensor_copy(
            vw[:, sc, :, 0:DH], pv[:].rearrange("p (h d) -> p h d", h=HG))
        yield

    def emit_qk_group(kind, mc, sq, pool, n=None):
        """one [128,512] psum group of the Q/K projection; generator-style"""
        dst = qwT if kind == "q" else kwT
        pp = pool.tile([128, 512], F32, tag="f")
        for dc in range(NDC):
            nc.tensor.matmul(
                pp[:], w_sbs[kind][:, dc, mc * 128:(mc + 1) * 128],
                xts["x" + kind][:, dc, sq * 512:(sq + 1) * 512],
                start=(dc == 0), stop=(dc == NDC - 1),
            )
            if n is not None:
                yield
        if kind == "q" and sq % 2 == 0:
            nc.scalar.copy(dst[:, mc, sq * 512:(sq + 1) * 512], pp[:])
        else:
            nc.vector.tensor_copy(dst[:, mc, sq * 512:(sq + 1) * 512], pp[:])
        if n is not None:
            yield

    for kind in ("q", "k"):
        for sq in range(4):
            for _ in emit_qk_group(kind, 0, sq, psum_p, n=None) or ():
                pass

    pctx.close()

    # ---------------- attention (+ projection fillers) ----------------
    actx = ExitStack()
    p_pool = actx.enter_context(tc.tile_pool(name="p", bufs=3))
    ot_pool = actx.enter_context(tc.tile_pool(name="ot", bufs=4))
    rq_pool = actx.enter_context(tc.tile_pool(name="rq", bufs=2))
    out_pool = actx.enter_context(tc.tile_pool(name="out", bufs=4))
    psum_s = actx.enter_context(tc.tile_pool(name="ps_s", bufs=2, space="PSUM"))
    psum_o = actx.enter_context(tc.tile_pool(name="ps_o", bufs=2, space="PSUM"))
    psum_f = actx.enter_context(tc.tile_pool(name="ps_f", bufs=2, space="PSUM"))

    def filler_gen():
        for sc in range(NSC):
            yield from emit_v_group(sc, psum_f)
        for mc in (1, 2, 3):
            for kind in ("q", "k"):
                for sq in range(4):
                    yield from emit_qk_group(kind, mc, sq, psum_f, n=1)

    fill = filler_gen()
    pulled = [0]
    UNITS_PER_SC = NDC + 1             # V: 8 matmuls + evac per s-chunk
    V_UNITS = NSC * UNITS_PER_SC
    UNITS_PER_MC = 2 * 4 * (NDC + 1)   # q+k, 4 sq-groups, 8 matmuls + evac

    def pull(n):
        for _ in range(n):
            if next(fill, "done") == "done":
                break
            pulled[0] += 1

    def pull_to(target):
        pull(max(0, target - pulled[0]))

    out_v = t["out"].ap().rearrange(
        "(a qb p) (hh d) -> a p qb hh d", a=NQH, p=128, hh=HG)

    pending_tail = [None]

    for hp in range(NMC):
        # V and the qwT/kwT chunk this head pair reads must be emitted first
        if hp > 0:
            pull_to(V_UNITS + UNITS_PER_MC * hp)
        kw_lo = kwT[0:64, hp, :]
        kw_hi = kwT[64:128, hp, :]
        qw_lo = qwT[0:64, hp, :]
        qw_hi = qwT[64:128, hp, :]
        for qh in range(NQH):
            q0 = qh * QH
            o_lo = psum_o.tile([DH + 1, QH], F32, tag="o", name=f"olo{hp}_{qh}")
            o_hi = psum_o.tile([DH + 1, QH], F32, tag="o", name=f"ohi{hp}_{qh}")

            def emit_s(kc):
                s_ps = psum_s.tile([128, 2, QH], F32, tag="s")
                nc.tensor.matmul(
                    s_ps[:, 0, :], kw_lo[:, kc * 128:(kc + 1) * 128],
                    qw_lo[:, q0:q0 + QH], start=True, stop=True)
                nc.tensor.matmul(
                    s_ps[:, 1, :], kw_hi[:, kc * 128:(kc + 1) * 128],
                    qw_hi[:, q0:q0 + QH], start=True, stop=True)
                return s_ps

            def emit_exp(s_ps):
                p_t = p_pool.tile([128, 2, QH], BF16, tag="p")
                nc.scalar.activation(
                    p_t[:], s_ps[:], EXP, bias=zbias[:], scale=0.125)
                return p_t

            def emit_av(kc, p_t):
                first, last = kc == 0, kc == NKC - 1
                nc.tensor.matmul(o_lo[:], vw[:, kc, 2 * hp, :], p_t[:, 0, :],
                                 start=first, stop=last)
                nc.tensor.matmul(o_hi[:], vw[:, kc, 2 * hp + 1, :], p_t[:, 1, :],
                                 start=first, stop=last)

            s_prev = emit_s(0)
            s_cur = emit_s(1)
            for kc in range(NKC):
                p_t = emit_exp(s_prev)
                if kc == 3 and pending_tail[0] is not None:
                    pending_tail[0]()
                    pending_tail[0] = None
                if hp == 0 and qh == 0:
                    # AV(kc) needs vw[:, kc] -> V s-chunk kc emitted by now
                    pull_to(UNITS_PER_SC * (kc + 1))
                else:
                    pull(N_FILL)
                emit_av(kc, p_t)
                s_prev = s_cur
                s_cur = emit_s(kc + 2) if kc + 2 < NKC else None
            if pending_tail[0] is not None:
                pending_tail[0]()

            def make_tail(hp=hp, qh=qh, o_lo=o_lo, o_hi=o_hi):
                def one(h, o_ps):
                    ot = ot_pool.tile([DH + 1, QH], F32, tag="ot",
                                      name=f"ot_{h}_{qh}")
                    nc.vector.tensor_copy(ot[:], o_ps[:])
                    tr = psum_f.tile([128, QB, DH + 1], F32, tag="f",
                                     name=f"tr_{h}_{qh}")
                    for qb in range(QB):
                        nc.tensor.transpose(
                            tr[:, qb, :], ot[:, qb * 128:(qb + 1) * 128],
                            ident[0:DH + 1, 0:DH + 1])
                    rq = rq_pool.tile([128, QB], F32, tag="rq",
                                      name=f"rq_{h}_{qh}")
                    nc.vector.reciprocal(rq[:], tr[:, :, DH])
                    nc.vector.tensor_mul(
                        rq[:], rq[:], qmaskT[:, qh * QB:(qh + 1) * QB])
                    ob = out_pool.tile([128, QB, DH], F32, tag="ob",
                                       name=f"ob_{h}_{qh}")
                    nc.vector.tensor_mul(
                        ob[:], tr[:, :, 0:DH], rq[:].broadcast_to([128, QB, DH]))
                    nc.sync.dma_start(out_v[qh][:, :, h, :], ob[:])

                def tail():
                    one(2 * hp, o_lo)
                    one(2 * hp + 1, o_hi)
                return tail

            pending_tail[0] = make_tail()

    pending_tail[0]()
    pull(10 ** 9)
    actx.close()
    ctx.close()


_BUILD_LOCK = threading.Lock()
_CACHE = {}


def _build():
    with _BUILD_LOCK:
        if "nc" in _CACHE:
            return _CACHE["nc"]
        nc = bacc.Bacc(
            "TRN2", target_bir_lowering=False, debug=False, num_devices=N_CORES
        )
        t = {
            "xq": nc.dram_tensor("xq", [D, S], BF16, kind="ExternalInput"),
            "xk": nc.dram_tensor("xk", [D, S], BF16, kind="ExternalInput"),
            "xv": nc.dram_tensor("xv", [D, S], BF16, kind="ExternalInput"),
            "wq": nc.dram_tensor("wq", [D, MC], BF16, kind="ExternalInput"),
            "wk": nc.dram_tensor("wk", [D, MC], BF16, kind="ExternalInput"),
            "wv": nc.dram_tensor("wv", [D, MC], BF16, kind="ExternalInput"),
            "vmaskT": nc.dram_tensor("vmaskT", [128, NKC], BF16,
                                     kind="ExternalInput"),
            "qmaskT": nc.dram_tensor("qmaskT", [128, NSC], F32,
                                     kind="ExternalInput"),
            "out": nc.dram_tensor("out", [S, MC], F32, kind="ExternalOutput"),
        }
        with tile.TileContext(nc) as tc:
            _emit(tc, t)
        nc.compile()
        _CACHE["nc"] = nc
        return nc


def _in_maps(q_value, k_value, v_value, v_mask, q_mask, Wq, Wk, Wv):
    bf = ml_dtypes.bfloat16
    xqt, xkt, xvt = {}, {}, {}
    for b in range(B):
        xqt[b] = np.ascontiguousarray(q_value[b].T.astype(bf))
        xkt[b] = np.ascontiguousarray(k_value[b].T.astype(bf))
        # fold key mask into the V rows (numerator side)
        xvt[b] = np.ascontiguousarray((v_value[b] * v_mask[b]).T.astype(bf))
    w8 = {}
    for g in range(2):
        m0 = g * MC
        w8[g] = (np.ascontiguousarray(Wq[:, m0:m0 + MC].astype(bf)),
                 np.ascontiguousarray(Wk[:, m0:m0 + MC].astype(bf)),
                 np.ascontiguousarray(Wv[:, m0:m0 + MC].astype(bf)))
    maps = []
    for c in range(N_CORES):
        b, g = c // 2, c % 2
        vm = v_mask[b, :, 0].reshape(NKC, 128).T
        qm = q_mask[b, :, 0].reshape(NSC, 128).T
        maps.append({
            "xq": xqt[b], "xk": xkt[b], "xv": xvt[b],
            "wq": w8[g][0], "wk": w8[g][1], "wv": w8[g][2],
            "vmaskT": np.ascontiguousarray(vm.astype(bf)),
            "qmaskT": np.ascontiguousarray(qm).astype(np.float32),
        })
    return maps


def _assemble(results):
    out = np.empty((B, S, HEADS * DH), dtype=np.float32)
    for c in range(N_CORES):
        b, g = c // 2, c % 2
        out[b, :, g * MC:(g + 1) * MC] = results[c]["out"]
    return out


def kernel(q_value, k_value, v_value, v_mask, q_mask, Wq, Wk, Wv,
           profile=False, trace_cores=None):
    nc = _build()
    maps = _in_maps(np.asarray(q_value, dtype=np.float32),
                    np.asarray(k_value, dtype=np.float32),
                    np.asarray(v_value, dtype=np.float32),
                    np.asarray(v_mask, dtype=np.float32),
                    np.asarray(q_mask, dtype=np.float32),
                    np.asarray(Wq, dtype=np.float32),
                    np.asarray(Wk, dtype=np.float32),
                    np.asarray(Wv, dtype=np.float32))
    if profile:
        _install_profile_hook()
    res = run_bass_kernel_spmd(
        nc, maps, list(range(N_CORES)),
        trace=profile, trace_cores=trace_cores,
    )
    out = _assemble(res.results)
    if profile:
        return out, res
    return out


def _install_profile_hook():
    """Wire up the NTFF profile hook that this container image lacks."""
    import types
    if "antenv.axon_hooks" in sys.modules:
        return
    try:
        from trn_agent_boot.trn_boot import _ntff_profile_via_ctypes
        hook = _ntff_profile_via_ctypes("/opt/axon/libaxon_pjrt.so")
    except Exception:
        hook = None
    mod = types.ModuleType("antenv.axon_hooks")
    mod.get_axon_ntff_profile_hook = lambda: hook
    sys.modules["antenv.axon_hooks"] = mod


if __name__ == "__main__":
    t0 = time.time()
    _build()
    print(f"build+compile: {time.time() - t0:.1f}s")


# revision 19
# speedup vs baseline: 1.5743x; 1.0231x over previous
"""Trainium2 Bass kernel for batched multi-head attention (v2, all-bf16).

Full module:  out = softmax((X_q Wq)(X_k Wk)^T / sqrt(dh) + keymask) (X_v Wv) * qmask
Shapes: B=4, S=2048, D=1024, H=16, dh=64.

Sharding over 8 NeuronCores: core c -> (batch b = c//2, head-group g = c%2).
Each core computes batch b, heads g*8..g*8+8 (Wq/Wk/Wv column-sharded by head).
No collectives; the host scatters inputs and gathers the [2048, 512] output
blocks into the full [4, 2048, 1024] output.

Host-side marshaling: X tensors are transposed (X^T, contraction dim on
partitions) and cast to bf16; W column blocks cast to bf16; v_mask is folded
into X_v rows (numerator) and shipped as vmaskT (denominator column). This
removes all on-chip PE transposes of X and their PSUM evacuations.

Per-core schedule (all matmuls bf16, moving N=512):
  Phase 0: V projection + Q/K projections for head pair 0 (mc=0).
  Attention, one head PAIR at a time (heads 2i/2i+1 live on partition halves
  0:64 / 64:128 of the mc=i chunk of QW^T/KW^T):
    per kc: S^T for both heads -> one [128, 2, 512] PSUM tile via two
    CONCURRENT K=64 matmuls on PE array row-tiles (0,0)/(64,0);
    one ScalarE exp (N=1024, bf16 out) covers both heads;
    two K=128 AV matmuls accumulate O^T[65, 512] per head (row 64 = sum of
    exp * v_mask = softmax denominator).
  The exp stream is the bottleneck (~1.1us per kc); leftover PE time inside
  the loop is filled with the NEXT head pair's Q/K projection matmuls
  (pulled from a generator), so projections cost almost no wall time.
  Tails (PE-transpose O^T, normalize by qmask/denom, DMA out) are deferred
  into the next iteration's stream.
"""

import os
import sys
import time
import threading

for _p in ("/opt/trn_rl_repo", "/opt/pypackages"):
    if _p not in sys.path and os.path.isdir(_p):
        sys.path.append(_p)

import numpy as np
import ml_dtypes
from contextlib import ExitStack

import concourse.bass as bass
import concourse.tile as tile
from concourse import bacc, mybir
from concourse.bass_utils import run_bass_kernel_spmd
from concourse.masks import make_identity

B, S, D = 4, 2048, 1024
HEADS, DH = 16, 64
N_CORES = 8
HG = HEADS // 2          # 8 heads per core
MC = HG * DH             # 512 output cols per core
NSC = S // 128           # 16 seq chunks
NDC = D // 128           # 8 contraction chunks
NMC = MC // 128          # 4 head-dim chunks (= head pairs)
NKC = NSC                # 16 key chunks

F32 = mybir.dt.float32
BF16 = mybir.dt.bfloat16
EXP = mybir.ActivationFunctionType.Exp

QH = 512                 # q-half size
NQH = S // QH
QB = QH // 128
N_FILL = int(os.environ.get("N_FILL", "2"))   # filler units pulled per kc


def _emit(tc, t):
    nc = tc.nc
    ctx = ExitStack()

    # ---------------- persistent pools / DMAs ----------------
    cpool = ctx.enter_context(tc.tile_pool(name="const", bufs=1))
    x_pool = ctx.enter_context(tc.tile_pool(name="x", bufs=1))
    w_pool = ctx.enter_context(tc.tile_pool(name="w", bufs=1))

    # W first (small, needed by the first projections), then X^T quarters in
    # consumption order: xq/xk (phase-0 Q/K mc0) before xv (filler V proj).
    w_sbs = {}
    for kind in ("q", "k", "v"):
        w_sb = w_pool.tile([128, NDC, MC], BF16, name="w" + kind, tag="w" + kind)
        nc.sync.dma_start(w_sb[:], t["w" + kind].ap().rearrange("(dc p) m -> p dc m", p=128))
        w_sbs[kind] = w_sb
    xts = {}
    for name in ("xq", "xk", "xv"):
        xt = x_pool.tile([128, NDC, S], BF16, name=name + "t", tag=name + "t")
        xts[name] = xt
    x_views = {name: t[name].ap().rearrange("(dc p) s -> p dc s", p=128)
               for name in ("xq", "xk", "xv")}
    # quarter DMAs ordered by first consumption: phase-0 Q/K mc0 needs
    # xq0/xk0; then V s-chunks (xv quarters) and K quarters feed head-pair
    # 0's k-loop; Q quarters are only needed from its second q-half on.
    dma_order = [("xq", 0), ("xk", 0), ("xv", 0), ("xv", 1), ("xk", 1),
                 ("xv", 2), ("xk", 2), ("xv", 3), ("xk", 3),
                 ("xq", 1), ("xq", 2), ("xq", 3)]
    for name, sq in dma_order:
        nc.sync.dma_start(xts[name][:, :, sq * 512:(sq + 1) * 512],
                          x_views[name][:, :, sq * 512:(sq + 1) * 512])

    ident = cpool.tile([128, 128], F32)
    make_identity(nc, ident[:])
    ident_b = cpool.tile([128, 128], BF16)
    nc.vector.tensor_copy(ident_b[:], ident[:])
    zbias = cpool.tile([128, 1], F32)
    nc.vector.memset(zbias[:], 0.0)
    qmaskT = cpool.tile([128, NSC], F32)
    nc.sync.dma_start(qmaskT[:], t["qmaskT"].ap())
    vmaskT = cpool.tile([128, NKC], BF16)
    nc.sync.dma_start(vmaskT[:], t["vmaskT"].ap())

    # exp table warmup while DMAs stream
    warm = cpool.tile([128, 1], BF16)
    nc.scalar.activation(warm[:], zbias[:], EXP, bias=zbias[:], scale=1.0)

    qk_pool = ctx.enter_context(tc.tile_pool(name="qk", bufs=1))
    qwT = qk_pool.tile([128, NMC, S], BF16)      # [m%128, mc, s]
    kwT = qk_pool.tile([128, NMC, S], BF16)
    vw = qk_pool.tile([128, NKC, HG, DH + 1], BF16)  # [k%128, kc, h, dh|m]
    nc.vector.tensor_copy(
        vw[:, :, :, DH:DH + 1],
        vmaskT[:].rearrange("p (k a b) -> p k a b", a=1, b=1)
        .broadcast_to([128, NKC, HG, 1]),
    )

    # ---------------- phase 0: Q/K mc=0 only ----------------
    pctx = ExitStack()
    psum_p = pctx.enter_context(tc.tile_pool(name="ps_p", bufs=2, space="PSUM"))

    def emit_v_group(sc, pool):
        """one [128,512] psum group of the V projection; generator-style"""
        pv = pool.tile([128, MC], F32, tag="f")
        for dc in range(NDC):
            nc.tensor.matmul(
                pv[:], xts["xv"][:, dc, sc * 128:(sc + 1) * 128],
                w_sbs["v"][:, dc, :],
                start=(dc == 0), stop=(dc == NDC - 1),
            )
            yield
        nc.vector.tensor_copy(
            vw[:, sc, :, 0:DH], pv[:].rearrange("p (h d) -> p h d", h=HG))
        yield

    def emit_qk_group(kind, mc, sq, pool, n=None):
        """one [128,512] psum group of the Q/K projection; generator-style"""
        dst = qwT if kind == "q" else kwT
        pp = pool.tile([128, 512], F32, tag="f")
        for dc in range(NDC):
            nc.tensor.matmul(
                pp[:], w_sbs[kind][:, dc, mc * 128:(mc + 1) * 128],
                xts["x" + kind][:, dc, sq * 512:(sq + 1) * 512],
                start=(dc == 0), stop=(dc == NDC - 1),
            )
            if n is not None:
                yield
        if kind == "q" and sq % 2 == 0:
            nc.scalar.copy(dst[:, mc, sq * 512:(sq + 1) * 512], pp[:])
        else:
            nc.vector.tensor_copy(dst[:, mc, sq * 512:(sq + 1) * 512], pp[:])
        if n is not None:
            yield

    for kind in ("q", "k"):
        for sq in range(4):
            for _ in emit_qk_group(kind, 0, sq, psum_p, n=None) or ():
                pass

    pctx.close()

    # ---------------- attention (+ projection fillers) ----------------
    actx = ExitStack()
    p_pool = actx.enter_context(tc.tile_pool(name="p", bufs=3))
    ot_pool = actx.enter_context(tc.tile_pool(name="ot", bufs=4))
    rq_pool = actx.enter_context(tc.tile_pool(name="rq", bufs=2))
    out_pool = actx.enter_context(tc.tile_pool(name="out", bufs=4))
    psum_s = actx.enter_context(tc.tile_pool(name="ps_s", bufs=2, space="PSUM"))
    psum_o = actx.enter_context(tc.tile_pool(name="ps_o", bufs=2, space="PSUM"))
    psum_f = actx.enter_context(tc.tile_pool(name="ps_f", bufs=2, space="PSUM"))

    def filler_gen():
        for sc in range(NSC):
            yield from emit_v_group(sc, psum_f)
        for mc in (1, 2, 3):
            for kind in ("q", "k"):
                for sq in range(4):
                    yield from emit_qk_group(kind, mc, sq, psum_f, n=1)

    fill = filler_gen()
    pulled = [0]
    UNITS_PER_SC = NDC + 1             # V: 8 matmuls + evac per s-chunk
    V_UNITS = NSC * UNITS_PER_SC
    UNITS_PER_MC = 2 * 4 * (NDC + 1)   # q+k, 4 sq-groups, 8 matmuls + evac

    def pull(n):
        for _ in range(n):
            if next(fill, "done") == "done":
                break
            pulled[0] += 1

    def pull_to(target):
        pull(max(0, target - pulled[0]))

    out_v = t["out"].ap().rearrange(
        "(a qb p) (hh d) -> a p qb hh d", a=NQH, p=128, hh=HG)

    def emit_s_for(hp, qh, kc):
        q0 = qh * QH
        s_ps = psum_s.tile([128, 2, QH], F32, tag="s")
        nc.tensor.matmul(
            s_ps[:, 0, :], kwT[0:64, hp, kc * 128:(kc + 1) * 128],
            qwT[0:64, hp, q0:q0 + QH], start=True, stop=True)
        nc.tensor.matmul(
            s_ps[:, 1, :], kwT[64:128, hp, kc * 128:(kc + 1) * 128],
            qwT[64:128, hp, q0:q0 + QH], start=True, stop=True)
        return s_ps

    def fill_target(hp):
        # fillers needed before head pair hp runs: V, then mc chunks 1..hp
        return V_UNITS + UNITS_PER_MC * hp if hp > 0 else 0

    pending_tail = [None]
    iters = [(hp, qh) for hp in range(NMC) for qh in range(NQH)]
    carry = []

    for it, (hp, qh) in enumerate(iters):
        o_lo = psum_o.tile([DH + 1, QH], F32, tag="o", name=f"olo{hp}_{qh}")
        o_hi = psum_o.tile([DH + 1, QH], F32, tag="o", name=f"ohi{hp}_{qh}")
        nxt = iters[it + 1] if it + 1 < len(iters) else None

        def emit_exp(s_ps):
            p_t = p_pool.tile([128, 2, QH], BF16, tag="p")
            nc.scalar.activation(
                p_t[:], s_ps[:], EXP, bias=zbias[:], scale=0.125)
            return p_t

        def emit_av(kc, p_t, hp=hp, o_lo=o_lo, o_hi=o_hi):
            first, last = kc == 0, kc == NKC - 1
            nc.tensor.matmul(o_lo[:], vw[:, kc, 2 * hp, :], p_t[:, 0, :],
                             start=first, stop=last)
            nc.tensor.matmul(o_hi[:], vw[:, kc, 2 * hp + 1, :], p_t[:, 1, :],
                             start=first, stop=last)

        if carry:
            s_prev, s_cur = carry
            carry = []
        else:
            pull_to(fill_target(hp))
            s_prev = emit_s_for(hp, qh, 0)
            s_cur = emit_s_for(hp, qh, 1)

        for kc in range(NKC):
            p_t = emit_exp(s_prev)
            if kc == 3 and pending_tail[0] is not None:
                pending_tail[0]()
                pending_tail[0] = None
            if hp == 0 and qh == 0:
                # AV(kc) needs vw[:, kc] -> V s-chunk kc emitted by now
                pull_to(UNITS_PER_SC * (kc + 1))
            else:
                pull(N_FILL)
            emit_av(kc, p_t)
            s_prev = s_cur
            if kc + 2 < NKC:
                s_cur = emit_s_for(hp, qh, kc + 2)
            elif nxt is not None:
                # pre-emit the next iteration's first S pairs to keep the
                # exp stream gapless across (hp, qh) boundaries
                if nxt[0] != hp:
                    pull_to(fill_target(nxt[0]))
                carry.append(emit_s_for(nxt[0], nxt[1], kc + 2 - NKC))
                s_cur = None

        if pending_tail[0] is not None:
            pending_tail[0]()

        def make_tail(hp=hp, qh=qh, o_lo=o_lo, o_hi=o_hi):
            def one(h, o_ps):
                ot = ot_pool.tile([DH + 1, QH], BF16, tag="ot",
                                  name=f"ot_{h}_{qh}")
                nc.vector.tensor_copy(ot[:], o_ps[:])
                tr = psum_f.tile([128, QB, DH + 2], BF16, tag="f",
                                 name=f"tr_{h}_{qh}")
                for qb in range(QB):
                    nc.tensor.transpose(
                        tr[:, qb, 0:DH + 1], ot[:, qb * 128:(qb + 1) * 128],
                        ident_b[0:DH + 1, 0:DH + 1])
                rq = rq_pool.tile([128, QB], F32, tag="rq",
                                  name=f"rq_{h}_{qh}")
                nc.vector.reciprocal(rq[:], tr[:, :, DH])
                nc.vector.tensor_mul(
                    rq[:], rq[:], qmaskT[:, qh * QB:(qh + 1) * QB])
                ob = out_pool.tile([128, QB, DH], F32, tag="ob",
                                   name=f"ob_{h}_{qh}")
                nc.vector.tensor_mul(
                    ob[:], tr[:, :, 0:DH], rq[:].broadcast_to([128, QB, DH]))
                nc.sync.dma_start(out_v[qh][:, :, h, :], ob[:])

            def tail():
                one(2 * hp, o_lo)
                one(2 * hp + 1, o_hi)
            return tail

        pending_tail[0] = make_tail()

    pending_tail[0]()
    pull(10 ** 9)
    actx.close()
    ctx.close()


_BUILD_LOCK = threading.Lock()
_CACHE = {}


def _build():
    with _BUILD_LOCK:
        if "nc" in _CACHE:
            return _CACHE["nc"]
        nc = bacc.Bacc(
            "TRN2", target_bir_lowering=False, debug=False, num_devices=N_CORES
        )
        t = {
            "xq": nc.dram_tensor("xq", [D, S], BF16, kind="ExternalInput"),
            "xk": nc.dram_tensor("xk", [D, S], BF16, kind="ExternalInput"),
            "xv": nc.dram_tensor("xv", [D, S], BF16, kind="ExternalInput"),
            "wq": nc.dram_tensor("wq", [D, MC], BF16, kind="ExternalInput"),
            "wk": nc.dram_tensor("wk", [D, MC], BF16, kind="ExternalInput"),
            "wv": nc.dram_tensor("wv", [D, MC], BF16, kind="ExternalInput"),
            "vmaskT": nc.dram_tensor("vmaskT", [128, NKC], BF16,
                                     kind="ExternalInput"),
            "qmaskT": nc.dram_tensor("qmaskT", [128, NSC], F32,
                                     kind="ExternalInput"),
            "out": nc.dram_tensor("out", [S, MC], F32, kind="ExternalOutput"),
        }
        with tile.TileContext(nc) as tc:
            _emit(tc, t)
        nc.compile()
        _CACHE["nc"] = nc
        return nc


def _in_maps(q_value, k_value, v_value, v_mask, q_mask, Wq, Wk, Wv):
    bf = ml_dtypes.bfloat16
    xqt, xkt, xvt = {}, {}, {}
    for b in range(B):
        xqt[b] = np.ascontiguousarray(q_value[b].T.astype(bf))
        xkt[b] = np.ascontiguousarray(k_value[b].T.astype(bf))
        # fold key mask into the V rows (numerator side)
        xvt[b] = np.ascontiguousarray((v_value[b] * v_mask[b]).T.astype(bf))
    w8 = {}
    for g in range(2):
        m0 = g * MC
        w8[g] = (np.ascontiguousarray(Wq[:, m0:m0 + MC].astype(bf)),
                 np.ascontiguousarray(Wk[:, m0:m0 + MC].astype(bf)),
                 np.ascontiguousarray(Wv[:, m0:m0 + MC].astype(bf)))
    maps = []
    for c in range(N_CORES):
        b, g = c // 2, c % 2
        vm = v_mask[b, :, 0].reshape(NKC, 128).T
        qm = q_mask[b, :, 0].reshape(NSC, 128).T
        maps.append({
            "xq": xqt[b], "xk": xkt[b], "xv": xvt[b],
            "wq": w8[g][0], "wk": w8[g][1], "wv": w8[g][2],
            "vmaskT": np.ascontiguousarray(vm.astype(bf)),
            "qmaskT": np.ascontiguousarray(qm).astype(np.float32),
        })
    return maps


def _assemble(results):
    out = np.empty((B, S, HEADS * DH), dtype=np.float32)
    for c in range(N_CORES):
        b, g = c // 2, c % 2
        out[b, :, g * MC:(g + 1) * MC] = results[c]["out"]
    return out


def kernel(q_value, k_value, v_value, v_mask, q_mask, Wq, Wk, Wv,
           profile=False, trace_cores=None):
    nc = _build()
    maps = _in_maps(np.asarray(q_value, dtype=np.float32),
                    np.asarray(k_value, dtype=np.float32),
                    np.asarray(v_value, dtype=np.float32),
                    np.asarray(v_mask, dtype=np.float32),
                    np.asarray(q_mask, dtype=np.float32),
                    np.asarray(Wq, dtype=np.float32),
                    np.asarray(Wk, dtype=np.float32),
                    np.asarray(Wv, dtype=np.float32))
    if profile:
        _install_profile_hook()
    res = run_bass_kernel_spmd(
        nc, maps, list(range(N_CORES)),
        trace=profile, trace_cores=trace_cores,
    )
    out = _assemble(res.results)
    if profile:
        return out, res
    return out


def _install_profile_hook():
    """Wire up the NTFF profile hook that this container image lacks."""
    import types
    if "antenv.axon_hooks" in sys.modules:
        return
    try:
        from trn_agent_boot.trn_boot import _ntff_profile_via_ctypes
        hook = _ntff_profile_via_ctypes("/opt/axon/libaxon_pjrt.so")
    except Exception:
        hook = None
    mod = types.ModuleType("antenv.axon_hooks")
    mod.get_axon_ntff_profile_hook = lambda: hook
    sys.modules["antenv.axon_hooks"] = mod


if __name__ == "__main__":
    t0 = time.time()
    _build()
    print(f"build+compile: {time.time() - t0:.1f}s")


# revision 23
# speedup vs baseline: 1.5910x; 1.0106x over previous
"""Trainium2 Bass kernel for batched multi-head attention (v2, all-bf16).

Full module:  out = softmax((X_q Wq)(X_k Wk)^T / sqrt(dh) + keymask) (X_v Wv) * qmask
Shapes: B=4, S=2048, D=1024, H=16, dh=64.

Sharding over 8 NeuronCores: core c -> (batch b = c//2, head-group g = c%2).
Each core computes batch b, heads g*8..g*8+8 (Wq/Wk/Wv column-sharded by head).
No collectives; the host scatters inputs and gathers the [2048, 512] output
blocks into the full [4, 2048, 1024] output.

Host-side marshaling: X tensors are transposed (X^T, contraction dim on
partitions) and cast to bf16; W column blocks cast to bf16; v_mask is folded
into X_v rows (numerator) and shipped as vmaskT (denominator column). This
removes all on-chip PE transposes of X and their PSUM evacuations.

Per-core schedule (all matmuls bf16, moving N=512):
  Phase 0: V projection + Q/K projections for head pair 0 (mc=0).
  Attention, one head PAIR at a time (heads 2i/2i+1 live on partition halves
  0:64 / 64:128 of the mc=i chunk of QW^T/KW^T):
    per kc: S^T for both heads -> one [128, 2, 512] PSUM tile via two
    CONCURRENT K=64 matmuls on PE array row-tiles (0,0)/(64,0);
    one ScalarE exp (N=1024, bf16 out) covers both heads;
    two K=128 AV matmuls accumulate O^T[65, 512] per head (row 64 = sum of
    exp * v_mask = softmax denominator).
  The exp stream is the bottleneck (~1.1us per kc); leftover PE time inside
  the loop is filled with the NEXT head pair's Q/K projection matmuls
  (pulled from a generator), so projections cost almost no wall time.
  Tails (PE-transpose O^T, normalize by qmask/denom, DMA out) are deferred
  into the next iteration's stream.
"""

import os
import sys
import time
import threading

for _p in ("/opt/trn_rl_repo", "/opt/pypackages"):
    if _p not in sys.path and os.path.isdir(_p):
        sys.path.append(_p)

import numpy as np
import ml_dtypes
from contextlib import ExitStack

import concourse.bass as bass
import concourse.tile as tile
from concourse import bacc, mybir
from concourse.bass_utils import run_bass_kernel_spmd
from concourse.masks import make_identity

B, S, D = 4, 2048, 1024
HEADS, DH = 16, 64
N_CORES = 8
HG = HEADS // 2          # 8 heads per core
MC = HG * DH             # 512 output cols per core
NSC = S // 128           # 16 seq chunks
NDC = D // 128           # 8 contraction chunks
NMC = MC // 128          # 4 head-dim chunks (= head pairs)
NKC = NSC                # 16 key chunks

F32 = mybir.dt.float32
BF16 = mybir.dt.bfloat16
EXP = mybir.ActivationFunctionType.Exp

QH = 512                 # q-half size
NQH = S // QH
QB = QH // 128
N_FILL = int(os.environ.get("N_FILL", "2"))   # filler units pulled per kc


def _emit(tc, t):
    nc = tc.nc
    ctx = ExitStack()

    # ---------------- persistent pools / DMAs ----------------
    cpool = ctx.enter_context(tc.tile_pool(name="const", bufs=1))
    x_pool = ctx.enter_context(tc.tile_pool(name="x", bufs=1))
    w_pool = ctx.enter_context(tc.tile_pool(name="w", bufs=1))

    # W first (small, needed by the first projections), then X^T quarters in
    # consumption order: xq/xk (phase-0 Q/K mc0) before xv (filler V proj).
    w_sbs = {}
    for kind in ("q", "k", "v"):
        w_sb = w_pool.tile([128, NDC, MC], BF16, name="w" + kind, tag="w" + kind)
        nc.sync.dma_start(w_sb[:], t["w" + kind].ap().rearrange("(dc p) m -> p dc m", p=128))
        w_sbs[kind] = w_sb
    xts = {}
    for name in ("xq", "xk", "xv"):
        xt = x_pool.tile([128, NDC, S], BF16, name=name + "t", tag=name + "t")
        xts[name] = xt
    x_views = {name: t[name].ap().rearrange("(dc p) s -> p dc s", p=128)
               for name in ("xq", "xk", "xv")}
    # quarter DMAs ordered by first consumption: phase-0 Q/K mc0 needs
    # xq0/xk0; then V s-chunks (xv quarters) and K quarters feed head-pair
    # 0's k-loop; Q quarters are only needed from its second q-half on.
    dma_order = [("xq", 0), ("xk", 0), ("xv", 0), ("xv", 1), ("xk", 1),
                 ("xv", 2), ("xk", 2), ("xv", 3), ("xk", 3),
                 ("xq", 1), ("xq", 2), ("xq", 3)]
    for name, sq in dma_order:
        nc.sync.dma_start(xts[name][:, :, sq * 512:(sq + 1) * 512],
                          x_views[name][:, :, sq * 512:(sq + 1) * 512])

    ident = cpool.tile([128, 128], F32)
    make_identity(nc, ident[:])
    ident_b = cpool.tile([128, 128], BF16)
    nc.vector.tensor_copy(ident_b[:], ident[:])
    zbias = cpool.tile([128, 1], F32)
    nc.vector.memset(zbias[:], 0.0)
    qmaskT = cpool.tile([128, NSC], F32)
    nc.sync.dma_start(qmaskT[:], t["qmaskT"].ap())
    vmaskT = cpool.tile([128, NKC], BF16)
    nc.sync.dma_start(vmaskT[:], t["vmaskT"].ap())

    # exp table warmup while DMAs stream
    warm = cpool.tile([128, 1], BF16)
    nc.scalar.activation(warm[:], zbias[:], EXP, bias=zbias[:], scale=1.0)

    qk_pool = ctx.enter_context(tc.tile_pool(name="qk", bufs=1))
    qwT = qk_pool.tile([128, NMC, S], BF16)      # [m%128, mc, s]
    kwT = qk_pool.tile([128, NMC, S], BF16)
    vw = qk_pool.tile([128, NKC, HG, DH + 1], BF16)  # [k%128, kc, h, dh|m]
    nc.vector.tensor_copy(
        vw[:, :, :, DH:DH + 1],
        vmaskT[:].rearrange("p (k a b) -> p k a b", a=1, b=1)
        .broadcast_to([128, NKC, HG, 1]),
    )

    # ---------------- phase 0: Q/K mc=0 only ----------------
    pctx = ExitStack()
    psum_p = pctx.enter_context(tc.tile_pool(name="ps_p", bufs=2, space="PSUM"))

    def emit_v_group(sc, pool):
        """one [128,512] psum group of the V projection; generator-style"""
        pv = pool.tile([128, MC], F32, tag="f")
        for dc in range(NDC):
            nc.tensor.matmul(
                pv[:], xts["xv"][:, dc, sc * 128:(sc + 1) * 128],
                w_sbs["v"][:, dc, :],
                start=(dc == 0), stop=(dc == NDC - 1),
            )
            yield
        nc.vector.tensor_copy(
            vw[:, sc, :, 0:DH], pv[:].rearrange("p (h d) -> p h d", h=HG))
        yield

    def emit_qk_group(kind, mc, sq, pool, n=None):
        """one [128,512] psum group of the Q/K projection; generator-style"""
        dst = qwT if kind == "q" else kwT
        pp = pool.tile([128, 512], F32, tag="f")
        for dc in range(NDC):
            nc.tensor.matmul(
                pp[:], w_sbs[kind][:, dc, mc * 128:(mc + 1) * 128],
                xts["x" + kind][:, dc, sq * 512:(sq + 1) * 512],
                start=(dc == 0), stop=(dc == NDC - 1),
            )
            if n is not None:
                yield
        if kind == "q" and sq % 2 == 0:
            nc.scalar.copy(dst[:, mc, sq * 512:(sq + 1) * 512], pp[:])
        else:
            nc.vector.tensor_copy(dst[:, mc, sq * 512:(sq + 1) * 512], pp[:])
        if n is not None:
            yield

    # only the sq=0 groups up front -- the first q-half's S matmuls need
    # just qwT[mc0, 0:512] and kwT[mc0, 0:128]; the rest arrives as fillers
    for kind in ("q", "k"):
        for _ in emit_qk_group(kind, 0, 0, psum_p, n=None) or ():
            pass

    pctx.close()

    # ---------------- attention (+ projection fillers) ----------------
    actx = ExitStack()
    p_pool = actx.enter_context(tc.tile_pool(name="p", bufs=3))
    ot_pool = actx.enter_context(tc.tile_pool(name="ot", bufs=4))
    rq_pool = actx.enter_context(tc.tile_pool(name="rq", bufs=2))
    out_pool = actx.enter_context(tc.tile_pool(name="out", bufs=4))
    psum_s = actx.enter_context(tc.tile_pool(name="ps_s", bufs=2, space="PSUM"))
    psum_o = actx.enter_context(tc.tile_pool(name="ps_o", bufs=2, space="PSUM"))
    psum_f = actx.enter_context(tc.tile_pool(name="ps_f", bufs=2, space="PSUM"))

    # filler schedule: named groups in deadline order.  V{sc} feeds
    # AV(hp0,qh0,kc=sc); K{j}/Q{j} are the remaining mc0 s-quarters feeding
    # hp0's S matmuls (K: k-chunks 4j.., Q: q-half j); mc1-3 feed later
    # head pairs.
    GSZ = NDC + 1                       # matmuls + evac per group
    prelude = ["V0", "V1", "K1", "V2", "V3", "V4", "K2", "V5", "V6", "V7",
               "K3", "V8", "Q1", "V9", "V10", "Q2", "V11", "V12", "Q3",
               "V13", "V14", "V15"]
    order = prelude + [f"{kind}{mc}_{sq}" for mc in (1, 2, 3)
                       for kind in ("q", "k") for sq in range(4)]
    end_pos = {name: GSZ * (i + 1) for i, name in enumerate(order)}

    def filler_gen():
        for name in order:
            if name.startswith("V"):
                yield from emit_v_group(int(name[1:]), psum_f)
            elif name.startswith("K") or name.startswith("Q"):
                yield from emit_qk_group(name[0].lower(), 0, int(name[1:]),
                                         psum_f, n=1)
            else:
                yield from emit_qk_group(name[0], int(name[1]), int(name[3:]),
                                         psum_f, n=1)

    fill = filler_gen()
    pulled = [0]
    PRELUDE_UNITS = GSZ * len(prelude)
    UNITS_PER_MC = 2 * 4 * GSZ

    def pull(n):
        for _ in range(n):
            if next(fill, "done") == "done":
                break
            pulled[0] += 1

    def pull_to(target):
        pull(max(0, target - pulled[0]))

    out_v = t["out"].ap().rearrange(
        "(a qb p) (hh d) -> a p qb hh d", a=NQH, p=128, hh=HG)

    def emit_s_for(hp, qh, kc):
        q0 = qh * QH
        s_ps = psum_s.tile([128, 2, QH], F32, tag="s")
        nc.tensor.matmul(
            s_ps[:, 0, :], kwT[0:64, hp, kc * 128:(kc + 1) * 128],
            qwT[0:64, hp, q0:q0 + QH], start=True, stop=True)
        nc.tensor.matmul(
            s_ps[:, 1, :], kwT[64:128, hp, kc * 128:(kc + 1) * 128],
            qwT[64:128, hp, q0:q0 + QH], start=True, stop=True)
        return s_ps

    def fill_target(hp):
        # fillers needed before head pair hp runs: prelude + mc chunks 1..hp
        return PRELUDE_UNITS + UNITS_PER_MC * hp if hp > 0 else 0

    pending_tail = [None]
    iters = [(hp, qh) for hp in range(NMC) for qh in range(NQH)]
    carry = []

    for it, (hp, qh) in enumerate(iters):
        o_lo = psum_o.tile([DH + 1, QH], F32, tag="o", name=f"olo{hp}_{qh}")
        o_hi = psum_o.tile([DH + 1, QH], F32, tag="o", name=f"ohi{hp}_{qh}")
        nxt = iters[it + 1] if it + 1 < len(iters) else None

        def emit_exp(s_ps):
            p_t = p_pool.tile([128, 2, QH], BF16, tag="p")
            nc.scalar.activation(
                p_t[:], s_ps[:], EXP, bias=zbias[:], scale=0.125)
            return p_t

        def emit_av(kc, p_t, hp=hp, o_lo=o_lo, o_hi=o_hi):
            first, last = kc == 0, kc == NKC - 1
            nc.tensor.matmul(o_lo[:], vw[:, kc, 2 * hp, :], p_t[:, 0, :],
                             start=first, stop=last)
            nc.tensor.matmul(o_hi[:], vw[:, kc, 2 * hp + 1, :], p_t[:, 1, :],
                             start=first, stop=last)

        if carry:
            s_prev, s_cur = carry
            carry = []
        else:
            pull_to(fill_target(hp))
            s_prev = emit_s_for(hp, qh, 0)
            s_cur = emit_s_for(hp, qh, 1)

        for kc in range(NKC):
            p_t = emit_exp(s_prev)
            if kc == 3 and pending_tail[0] is not None:
                pending_tail[0]()
                pending_tail[0] = None
            if hp == 0 and qh == 0:
                # AV(kc) needs V s-chunk kc; S(kc+2) below needs K quarter
                tgt = end_pos[f"V{kc}"]
                if kc + 2 < NKC and (kc + 2) // 4 > 0:
                    tgt = max(tgt, end_pos[f"K{(kc + 2) // 4}"])
                pull_to(tgt)
            else:
                pull(N_FILL)
            emit_av(kc, p_t)
            s_prev = s_cur
            if kc + 2 < NKC:
                s_cur = emit_s_for(hp, qh, kc + 2)
            elif nxt is not None:
                # pre-emit the next iteration's first S pairs to keep the
                # exp stream gapless across (hp, qh) boundaries
                if nxt[0] != hp:
                    pull_to(fill_target(nxt[0]))
                elif nxt[0] == 0 and nxt[1] >= 1:
                    pull_to(end_pos[f"Q{nxt[1]}"])
                carry.append(emit_s_for(nxt[0], nxt[1], kc + 2 - NKC))
                s_cur = None

        if pending_tail[0] is not None:
            pending_tail[0]()

        def make_tail(hp=hp, qh=qh, o_lo=o_lo, o_hi=o_hi):
            def one(h, o_ps):
                ot = ot_pool.tile([DH + 1, QH], BF16, tag="ot",
                                  name=f"ot_{h}_{qh}")
                nc.vector.tensor_copy(ot[:], o_ps[:])
                tr = psum_f.tile([128, QB, DH + 2], BF16, tag="f",
                                 name=f"tr_{h}_{qh}")
                for qb in range(QB):
                    nc.tensor.transpose(
                        tr[:, qb, 0:DH + 1], ot[:, qb * 128:(qb + 1) * 128],
                        ident_b[0:DH + 1, 0:DH + 1])
                rq = rq_pool.tile([128, QB], F32, tag="rq",
                                  name=f"rq_{h}_{qh}")
                nc.vector.reciprocal(rq[:], tr[:, :, DH])
                nc.vector.tensor_mul(
                    rq[:], rq[:], qmaskT[:, qh * QB:(qh + 1) * QB])
                ob = out_pool.tile([128, QB, DH], F32, tag="ob",
                                   name=f"ob_{h}_{qh}")
                nc.vector.tensor_mul(
                    ob[:], tr[:, :, 0:DH], rq[:].broadcast_to([128, QB, DH]))
                nc.sync.dma_start(out_v[qh][:, :, h, :], ob[:])

            def tail():
                one(2 * hp, o_lo)
                one(2 * hp + 1, o_hi)
            return tail

        pending_tail[0] = make_tail()

    pending_tail[0]()
    pull(10 ** 9)
    actx.close()
    ctx.close()


_BUILD_LOCK = threading.Lock()
_CACHE = {}


def _build():
    with _BUILD_LOCK:
        if "nc" in _CACHE:
            return _CACHE["nc"]
        nc = bacc.Bacc(
            "TRN2", target_bir_lowering=False, debug=False, num_devices=N_CORES
        )
        t = {
            "xq": nc.dram_tensor("xq", [D, S], BF16, kind="ExternalInput"),
            "xk": nc.dram_tensor("xk", [D, S], BF16, kind="ExternalInput"),
            "xv": nc.dram_tensor("xv", [D, S], BF16, kind="ExternalInput"),
            "wq": nc.dram_tensor("wq", [D, MC], BF16, kind="ExternalInput"),
            "wk": nc.dram_tensor("wk", [D, MC], BF16, kind="ExternalInput"),
            "wv": nc.dram_tensor("wv", [D, MC], BF16, kind="ExternalInput"),
            "vmaskT": nc.dram_tensor("vmaskT", [128, NKC], BF16,
                                     kind="ExternalInput"),
            "qmaskT": nc.dram_tensor("qmaskT", [128, NSC], F32,
                                     kind="ExternalInput"),
            "out": nc.dram_tensor("out", [S, MC], F32, kind="ExternalOutput"),
        }
        with tile.TileContext(nc) as tc:
            _emit(tc, t)
        nc.compile()
        _CACHE["nc"] = nc
        return nc


def _in_maps(q_value, k_value, v_value, v_mask, q_mask, Wq, Wk, Wv):
    bf = ml_dtypes.bfloat16
    xqt, xkt, xvt = {}, {}, {}
    for b in range(B):
        xqt[b] = np.ascontiguousarray(q_value[b].T.astype(bf))
        xkt[b] = np.ascontiguousarray(k_value[b].T.astype(bf))
        # fold key mask into the V rows (numerator side)
        xvt[b] = np.ascontiguousarray((v_value[b] * v_mask[b]).T.astype(bf))
    w8 = {}
    for g in range(2):
        m0 = g * MC
        w8[g] = (np.ascontiguousarray(Wq[:, m0:m0 + MC].astype(bf)),
                 np.ascontiguousarray(Wk[:, m0:m0 + MC].astype(bf)),
                 np.ascontiguousarray(Wv[:, m0:m0 + MC].astype(bf)))
    maps = []
    for c in range(N_CORES):
        b, g = c // 2, c % 2
        vm = v_mask[b, :, 0].reshape(NKC, 128).T
        qm = q_mask[b, :, 0].reshape(NSC, 128).T
        maps.append({
            "xq": xqt[b], "xk": xkt[b], "xv": xvt[b],
            "wq": w8[g][0], "wk": w8[g][1], "wv": w8[g][2],
            "vmaskT": np.ascontiguousarray(vm.astype(bf)),
            "qmaskT": np.ascontiguousarray(qm).astype(np.float32),
        })
    return maps


def _assemble(results):
    out = np.empty((B, S, HEADS * DH), dtype=np.float32)
    for c in range(N_CORES):
        b, g = c // 2, c % 2
        out[b, :, g * MC:(g + 1) * MC] = results[c]["out"]
    return out


def kernel(q_value, k_value, v_value, v_mask, q_mask, Wq, Wk, Wv,
           profile=False, trace_cores=None):
    nc = _build()
    maps = _in_maps(np.asarray(q_value, dtype=np.float32),
                    np.asarray(k_value, dtype=np.float32),
                    np.asarray(v_value, dtype=np.float32),
                    np.asarray(v_mask, dtype=np.float32),
                    np.asarray(q_mask, dtype=np.float32),
                    np.asarray(Wq, dtype=np.float32),
                    np.asarray(Wk, dtype=np.float32),
                    np.asarray(Wv, dtype=np.float32))
    if profile:
        _install_profile_hook()
    res = run_bass_kernel_spmd(
        nc, maps, list(range(N_CORES)),
        trace=profile, trace_cores=trace_cores,
    )
    out = _assemble(res.results)
    if profile:
        return out, res
    return out


def _install_profile_hook():
    """Wire up the NTFF profile hook that this container image lacks."""
    import types
    if "antenv.axon_hooks" in sys.modules:
        return
    try:
        from trn_agent_boot.trn_boot import _ntff_profile_via_ctypes
        hook = _ntff_profile_via_ctypes("/opt/axon/libaxon_pjrt.so")
    except Exception:
        hook = None
    mod = types.ModuleType("antenv.axon_hooks")
    mod.get_axon_ntff_profile_hook = lambda: hook
    sys.modules["antenv.axon_hooks"] = mod


if __name__ == "__main__":
    t0 = time.time()
    _build()
    print(f"build+compile: {time.time() - t0:.1f}s")
